# revision 3
# baseline (speedup 1.0000x reference)
"""BitLinear (RMSNorm + ternary-quantized matmul) TRN2 kernel.

Computation (reference semantics):
    x_norm = x * rsqrt(mean(x^2, -1) + 1e-6) * gamma          [B,S,Din]
    scale  = max(mean(|weight|), 1e-5)                        scalar
    wq     = round(clip(weight/scale, -1, 1))  in {-1,0,1}    [Dout,Din]
    out    = (x_norm @ wq.T) * scale                          [B,S,Dout]

Distribution strategy (8 NeuronCores, full inputs in / full output out):
  Token-parallel: each core takes T/8 = 1024 tokens of x, the full
  (host-pre-transposed) weight, and produces the full 8192 output features
  for its tokens.  The global |w|-mean reduction is computed on-device by a
  small first kernel where each core reduces 1/8 of the weight; the eight
  partial sums are combined on host into the scalar `scale` fed to the main
  kernel.  round(clip(w/scale)) with round-half-even is implemented exactly
  as (w > 0.5*scale) - (w < -0.5*scale).

  Main kernel per core: RMSNorm in fp32, PE-transpose of x_norm to [k,t]
  layout, cast to fp16 (weights are exact ternary in fp16), dense fp16
  matmul accumulating in fp32 PSUM over 16 k-tiles.
"""

import os
import sys

sys.path.insert(0, "/opt/trn_rl_repo")

import numpy as np

N_CORES = 8
B, S, D_IN, D_OUT = 4, 2048, 2048, 8192
T = B * S                    # 8192 tokens
TPC = T // N_CORES           # 1024 tokens per core
P = 128
KO = D_IN // P               # 16 k-tiles
NT = TPC // P                # 8 token tiles per core
OC = 512                     # output-feature chunk (one PSUM bank)
NOC = D_OUT // OC            # 16 chunks
KH = KO // 2                 # quantize the weight chunk in 2 k-halves
EPS_RMS = 1e-6
EPS_SCALE = 1e-5

_BUILT = {}
LAST_PROFILE = {}


def _legalize_waits(nc):
    """Split multi-wait sync_info into preceding single-wait NOPs.

    The walrus build in this container caps embedded sync waits at 1 per
    instruction (2 for EventSemaphore); Tile's kernel-tail drain exceeds it.
    """
    from concourse import mybir

    n_fixed = 0
    for bb in nc.main_func.blocks:
        out = []
        changed = False
        for inst in bb.instructions:
            si = inst.sync_info
            waits = list(si.on_wait) if si is not None and si.on_wait else []
            cap = 2 if isinstance(inst, mybir.InstEventSemaphore) else 1
            if len(waits) > cap:
                for w in waits[:-cap]:
                    out.append(
                        mybir.InstNoOp(
                            name=f"{inst.name}-ws{n_fixed}",
                            engine=inst.engine,
                            sync_info=mybir.SyncInfo(on_wait=[w], on_update=[]),
                            text_hint="waitsplit",
                            bass_nofuse=True,
                        )
                    )
                    n_fixed += 1
                si.on_wait = waits[-cap:]
                changed = True
            out.append(inst)
        if changed:
            bb.instructions = out
    return n_fixed


def _build_scale_kernel():
    """Per-core partial sum of |w| over a [D_OUT/8, D_IN] row-slice of weight."""
    import concourse.bass as bass
    import concourse.tile as tile
    from concourse import mybir

    f32 = mybir.dt.float32
    ALU = mybir.AluOpType
    ROWS = D_OUT // N_CORES          # 1024
    NTILES = ROWS // P               # 8

    nc = bass.Bass()
    w_in = nc.dram_tensor("ws", [ROWS, D_IN], f32, kind="ExternalInput")
    p_out = nc.dram_tensor("partials", [P, NTILES], f32, kind="ExternalOutput")

    with tile.TileContext(nc) as tc:
        with (
            tc.tile_pool(name="wp", bufs=3) as wp,
            tc.tile_pool(name="acc", bufs=1) as accp,
        ):
            acc = accp.tile([P, NTILES], f32)
            w3 = w_in.rearrange("(n p) k -> n p k", p=P)
            for i in range(NTILES):
                t = wp.tile([P, D_IN], f32)
                nc.sync.dma_start(t[:], w3[i])
                nc.vector.tensor_reduce(
                    acc[:, i : i + 1],
                    t[:],
                    axis=mybir.AxisListType.X,
                    op=ALU.add,
                    apply_absolute_value=True,
                )
            nc.sync.dma_start(p_out[:], acc[:])
    _legalize_waits(nc)
    return nc


def _build_main_kernel():
    import concourse.bass as bass
    import concourse.tile as tile
    from concourse import mybir
    from concourse.masks import make_identity

    f32 = mybir.dt.float32
    fp16 = mybir.dt.float16
    AF = mybir.ActivationFunctionType
    ALU = mybir.AluOpType

    nc = bass.Bass()
    x_in = nc.dram_tensor("x", [TPC, D_IN], f32, kind="ExternalInput")
    wt_in = nc.dram_tensor("wt", [D_IN, D_OUT], f32, kind="ExternalInput")
    g_in = nc.dram_tensor("gamma", [D_IN], f32, kind="ExternalInput")
    s_in = nc.dram_tensor("scalars", [2], f32, kind="ExternalInput")  # [scale, tau]
    out = nc.dram_tensor("out", [TPC, D_OUT], f32, kind="ExternalOutput")

    with tile.TileContext(nc) as tc:
        with (
            tc.tile_pool(name="singles", bufs=1) as singles,
            tc.tile_pool(name="xp", bufs=2) as xp,
            tc.tile_pool(name="sq", bufs=1) as sqp,
            tc.tile_pool(name="stats", bufs=2) as stats,
            tc.tile_pool(name="wraw", bufs=2) as wrawp,
            tc.tile_pool(name="wm", bufs=2) as wmp,
            tc.tile_pool(name="wq", bufs=2) as wqp,
            tc.tile_pool(name="op", bufs=4) as op,
            tc.tile_pool(name="tps", bufs=2, space="PSUM") as tps,
            tc.tile_pool(name="mps", bufs=2, space="PSUM") as mps,
        ):
            # ---- constants ----
            ident = singles.tile([P, P], f32)
            make_identity(nc, ident)
            eps_t = singles.tile([P, 1], f32)
            nc.vector.memset(eps_t[:], EPS_RMS)
            scale_sb = singles.tile([P, 1], f32)
            nc.sync.dma_start(scale_sb[:], s_in[0:1].to_broadcast((P, 1)))
            tau_sb = singles.tile([P, 1], f32)
            nc.sync.dma_start(tau_sb[:], s_in[1:2].to_broadcast((P, 1)))
            ntau_sb = singles.tile([P, 1], f32)
            nc.vector.tensor_scalar_mul(ntau_sb[:], tau_sb[:], -1.0)
            gamma_sb = singles.tile([P, KO], f32)
            nc.sync.dma_start(gamma_sb[:], g_in.rearrange("(ko p) -> p ko", p=P))
            # gs[p, ko] = gamma[ko*128+p] * scale  (folded into x_norm^T)
            gs = singles.tile([P, KO], f32)
            nc.vector.tensor_scalar_mul(gs[:], gamma_sb[:], scale_sb[:, 0:1])

            # x_norm^T, fp16, [k-part, ko, t] resident for the whole kernel
            xnT = singles.tile([P, KO, TPC], fp16)

            # ---- phase A: RMSNorm + transpose ----
            for t in range(NT):
                xt = xp.tile([P, D_IN], f32)
                nc.sync.dma_start(xt[:], x_in[t * P : (t + 1) * P, :])
                sq = sqp.tile([P, D_IN], f32)
                ss = stats.tile([P, 1], f32)
                nc.scalar.activation(sq[:], xt[:], AF.Square, accum_out=ss[:, 0:1])
                rms = stats.tile([P, 1], f32)
                nc.scalar.activation(
                    rms[:], ss[:, 0:1], AF.Sqrt, scale=1.0 / D_IN, bias=eps_t[:, 0:1]
                )
                inv = stats.tile([P, 1], f32)
                nc.vector.reciprocal(inv[:], rms[:])
                xn = xp.tile([P, D_IN], f32)
                nc.vector.tensor_scalar_mul(xn[:], xt[:], inv[:, 0:1])
                for ko in range(KO):
                    ptr = tps.tile([P, P], f32)
                    nc.tensor.transpose(ptr[:], xn[:, ko * P : (ko + 1) * P], ident[:])
                    # cast to fp16 multiplying by gamma*scale (per-k scalar)
                    nc.vector.tensor_scalar(
                        xnT[:, ko, t * P : (t + 1) * P],
                        ptr[:],
                        gs[:, ko : ko + 1],
                        None,
                        op0=ALU.mult,
                    )

            # ---- phase B: stream weight chunks, quantize, matmul ----
            wt3 = wt_in.rearrange("(ko p) o -> p ko o", p=P)  # [128, 16, 8192]
            for oc in range(NOC):
                osl = slice(oc * OC, (oc + 1) * OC)
                wq = wqp.tile([P, KO, OC], fp16)
                for h in range(2):
                    ksl = slice(h * KH, (h + 1) * KH)
                    wr = wrawp.tile([P, KH, OC], f32)
                    nc.sync.dma_start(wr[:], wt3[:, ksl, osl])
                    m1 = wmp.tile([P, KH, OC], fp16)
                    nc.vector.tensor_scalar(
                        m1[:], wr[:], tau_sb[:, 0:1], None, op0=ALU.is_gt
                    )
                    m2 = wmp.tile([P, KH, OC], fp16)
                    nc.gpsimd.tensor_scalar(
                        m2[:], wr[:], ntau_sb[:, 0:1], None, op0=ALU.is_lt
                    )
                    nc.vector.tensor_tensor(
                        wq[:, ksl, :], m1[:], m2[:], op=ALU.subtract
                    )
                for t in range(NT):
                    ps = mps.tile([P, OC], f32)
                    for ko in range(KO):
                        nc.tensor.matmul(
                            ps[:],
                            xnT[:, ko, t * P : (t + 1) * P],
                            wq[:, ko, :],
                            start=(ko == 0),
                            stop=(ko == KO - 1),
                        )
                    ot = op.tile([P, OC], f32)
                    nc.vector.tensor_copy(ot[:], ps[:])
                    nc.sync.dma_start(out[t * P : (t + 1) * P, osl], ot[:])

    _legalize_waits(nc)
    return nc


def _ensure_ntff_hook():
    """Provide antenv.axon_hooks (missing from this image) so that
    run_bass_kernel_spmd(trace=True) can reach the libaxon NTFF profiler."""
    import types

    try:
        from antenv.axon_hooks import get_axon_ntff_profile_hook  # noqa: F401

        return True
    except ImportError:
        pass
    try:
        import antenv
        from trn_agent_boot.trn_boot import _ntff_profile_via_ctypes

        hook = _ntff_profile_via_ctypes("/opt/axon/libaxon_pjrt.so")
        mod = types.ModuleType("antenv.axon_hooks")
        _state = {"hook": hook}
        mod.set_axon_ntff_profile_hook = lambda h: _state.__setitem__("hook", h)
        mod.get_axon_ntff_profile_hook = lambda: _state["hook"]
        sys.modules["antenv.axon_hooks"] = mod
        antenv.axon_hooks = mod
        return hook is not None
    except Exception:
        return False


def _run(nc, in_maps, trace, tag):
    from concourse.bass_utils import run_bass_kernel_spmd

    kwargs = {}
    if trace and _ensure_ntff_hook():
        kwargs = dict(trace=True, trace_cores=list(range(N_CORES)))
        base = os.environ.get("BASS_PROBLEM_TRACE_DIR")
        if base:
            tdir = os.path.join(base, tag)
            os.makedirs(tdir, exist_ok=True)
            kwargs["tmpdir"] = tdir
    try:
        res = run_bass_kernel_spmd(nc, in_maps, list(range(N_CORES)), **kwargs)
    except Exception:
        if not kwargs:
            raise
        # tracing path failed; fall back to a plain run
        res = run_bass_kernel_spmd(nc, in_maps, list(range(N_CORES)))
    if trace:
        LAST_PROFILE[tag] = {
            "exec_time_ns": res.exec_time_ns,
            "mean_exec_time_ns": res.mean_exec_time_ns,
        }
    return res.results


def kernel(x, weight, gamma):
    trace = bool(int(os.environ.get("BASS_PROBLEM_TRACE", "0")))

    x = np.ascontiguousarray(np.asarray(x, dtype=np.float32))
    weight = np.ascontiguousarray(np.asarray(weight, dtype=np.float32))
    gamma = np.ascontiguousarray(np.asarray(gamma, dtype=np.float32))
    assert x.shape == (B, S, D_IN) and weight.shape == (D_OUT, D_IN)

    if "k1" not in _BUILT:
        _BUILT["k1"] = _build_scale_kernel()
    if "k2" not in _BUILT:
        _BUILT["k2"] = _build_main_kernel()

    # --- kernel 1: global mean(|w|) partials, 1/8 of the weight per core ---
    rows = D_OUT // N_CORES
    in1 = [
        {"ws": weight[c * rows : (c + 1) * rows]} for c in range(N_CORES)
    ]
    res1 = _run(_BUILT["k1"], in1, trace, "k1")
    total = np.float64(0.0)
    for c in range(N_CORES):
        total += res1[c]["partials"].astype(np.float64).sum()
    scale = np.float32(max(total / (D_OUT * D_IN), EPS_SCALE))
    tau = np.float32(0.5) * scale
    scalars = np.array([scale, tau], dtype=np.float32)

    # --- kernel 2: RMSNorm + quantized matmul, token-parallel ---
    x_flat = x.reshape(T, D_IN)
    wT = np.ascontiguousarray(weight.T)
    in2 = [
        {
            "x": x_flat[c * TPC : (c + 1) * TPC],
            "wt": wT,
            "gamma": gamma,
            "scalars": scalars,
        }
        for c in range(N_CORES)
    ]
    res2 = _run(_BUILT["k2"], in2, trace, "k2")
    out = np.concatenate([res2[c]["out"] for c in range(N_CORES)], axis=0)
    return out.reshape(B, S, D_OUT)


# revision 10
# speedup vs baseline: 4.4571x; 4.4571x over previous
"""BitLinear (RMSNorm + ternary-quantized matmul) TRN2 kernel.

Computation (reference semantics):
    x_norm = x * rsqrt(mean(x^2, -1) + 1e-6) * gamma          [B,S,Din]
    scale  = max(mean(|weight|), 1e-5)                        scalar
    wq     = round(clip(weight/scale, -1, 1))  in {-1,0,1}    [Dout,Din]
    out    = (x_norm @ wq.T) * scale                          [B,S,Dout]

Distribution strategy (8 NeuronCores, full inputs in / full output out):
  Token-parallel: each core takes T/8 = 1024 tokens of x, the full
  (host-pre-transposed) weight, and produces the full 8192 output features
  for its tokens.  The global |w|-mean reduction is computed on-device by a
  small first kernel where each core reduces 1/8 of the weight; the eight
  partial sums are combined on host into the scalar `scale` fed to the main
  kernel.  round(clip(w/scale)) with round-half-even is implemented exactly
  as (w > 0.5*scale) - (w < -0.5*scale).

  Main kernel per core: RMSNorm in fp32, PE-transpose of x_norm to [k,t]
  layout, cast to fp16 (weights are exact ternary in fp16), dense fp16
  matmul accumulating in fp32 PSUM over 16 k-tiles.
"""

import os
import sys

sys.path.insert(0, "/opt/trn_rl_repo")

import numpy as np

N_CORES = 8
B, S, D_IN, D_OUT = 4, 2048, 2048, 8192
T = B * S                    # 8192 tokens
TPC = T // N_CORES           # 1024 tokens per core
P = 128
KO = D_IN // P               # 16 k-tiles
NT = TPC // P                # 8 token tiles per core
OC = 512                     # output-feature chunk (one PSUM bank)
NOC = D_OUT // OC            # 16 chunks
KH = KO // 2                 # quantize the weight chunk in 2 k-halves
EPS_RMS = 1e-6
EPS_SCALE = 1e-5

_BUILT = {}
LAST_PROFILE = {}


def _legalize_waits(nc):
    """Split multi-wait sync_info into preceding single-wait NOPs.

    The walrus build in this container caps embedded sync waits at 1 per
    instruction (2 for EventSemaphore); Tile's kernel-tail drain exceeds it.
    """
    from concourse import mybir

    n_fixed = 0
    for bb in nc.main_func.blocks:
        out = []
        changed = False
        for inst in bb.instructions:
            si = inst.sync_info
            waits = list(si.on_wait) if si is not None and si.on_wait else []
            cap = 2 if isinstance(inst, mybir.InstEventSemaphore) else 1
            if len(waits) > cap:
                for w in waits[:-cap]:
                    out.append(
                        mybir.InstNoOp(
                            name=f"{inst.name}-ws{n_fixed}",
                            engine=inst.engine,
                            sync_info=mybir.SyncInfo(on_wait=[w], on_update=[]),
                            text_hint="waitsplit",
                            bass_nofuse=True,
                        )
                    )
                    n_fixed += 1
                si.on_wait = waits[-cap:]
                changed = True
            out.append(inst)
        if changed:
            bb.instructions = out
    return n_fixed


def _build_scale_kernel():
    """Per-core partial sum of |w| over a [D_OUT/8, D_IN] row-slice of weight."""
    import concourse.bass as bass
    import concourse.tile as tile
    from concourse import mybir

    f32 = mybir.dt.float32
    ALU = mybir.AluOpType
    ROWS = D_OUT // N_CORES          # 1024
    NTILES = ROWS // P               # 8

    nc = bass.Bass()
    w_in = nc.dram_tensor("ws", [ROWS, D_IN], f32, kind="ExternalInput")
    p_out = nc.dram_tensor("partials", [P, NTILES], f32, kind="ExternalOutput")

    with tile.TileContext(nc) as tc:
        with (
            tc.tile_pool(name="wp", bufs=3) as wp,
            tc.tile_pool(name="acc", bufs=1) as accp,
        ):
            acc = accp.tile([P, NTILES], f32)
            w3 = w_in.rearrange("(n p) k -> n p k", p=P)
            for i in range(NTILES):
                t = wp.tile([P, D_IN], f32)
                nc.sync.dma_start(t[:], w3[i])
                nc.vector.tensor_reduce(
                    acc[:, i : i + 1],
                    t[:],
                    axis=mybir.AxisListType.X,
                    op=ALU.add,
                    apply_absolute_value=True,
                )
            nc.sync.dma_start(p_out[:], acc[:])
    _legalize_waits(nc)
    return nc


def _build_main_kernel():
    import concourse.bass as bass
    import concourse.tile as tile
    from concourse import mybir
    from concourse.masks import make_identity

    f32 = mybir.dt.float32
    fp16 = mybir.dt.float16
    AF = mybir.ActivationFunctionType
    ALU = mybir.AluOpType

    nc = bass.Bass()
    x_in = nc.dram_tensor("x", [TPC, D_IN], f32, kind="ExternalInput")
    wt_in = nc.dram_tensor("wt", [D_IN, D_OUT], f32, kind="ExternalInput")
    g_in = nc.dram_tensor("gamma", [D_IN], f32, kind="ExternalInput")
    s_in = nc.dram_tensor("scalars", [1], f32, kind="ExternalInput")  # [tau]
    out = nc.dram_tensor("out", [TPC, D_OUT], f32, kind="ExternalOutput")

    with tile.TileContext(nc) as tc:
        with (
            tc.tile_pool(name="singles", bufs=1) as singles,
            tc.tile_pool(name="xp", bufs=2) as xp,
            tc.tile_pool(name="sq", bufs=1) as sqp,
            tc.tile_pool(name="stats", bufs=2) as stats,
            tc.tile_pool(name="wraw", bufs=2) as wrawp,
            tc.tile_pool(name="wm", bufs=2) as wmp,
            tc.tile_pool(name="wq", bufs=2) as wqp,
            tc.tile_pool(name="op", bufs=4) as op,
            tc.tile_pool(name="tps", bufs=2, space="PSUM") as tps,
            tc.tile_pool(name="mps", bufs=4, space="PSUM") as mps,
        ):
            # ---- constants ----
            ident = singles.tile([P, P], f32)
            make_identity(nc, ident)
            eps_t = singles.tile([P, 1], f32)
            nc.vector.memset(eps_t[:], EPS_RMS)
            tau_sb = singles.tile([P, 1], f32)
            nc.sync.dma_start(tau_sb[:], s_in[0:1].to_broadcast((P, 1)))
            ntau_sb = singles.tile([P, 1], f32)
            nc.vector.tensor_scalar_mul(ntau_sb[:], tau_sb[:], -1.0)
            gamma_sb = singles.tile([P, KO], f32)
            nc.sync.dma_start(gamma_sb[:], g_in.rearrange("(ko p) -> p ko", p=P))
            # Quantized weights are kept as 2*wq = sign(w-tau)+sign(w+tau) in
            # {-2,0,2}; the compensating 1/2 (and the global `scale` and gamma)
            # are folded into x_norm^T:  gs[p,ko] = gamma[ko*128+p] * scale/2
            # and tau == scale/2 exactly.
            gs = singles.tile([P, KO], f32)
            nc.vector.tensor_scalar_mul(gs[:], gamma_sb[:], tau_sb[:, 0:1])

            # x_norm^T, fp16, [k-part, ko, t] resident for the whole kernel
            xnT = singles.tile([P, KO, TPC], fp16)

            # ---- phase A: RMSNorm + transpose ----
            for t in range(NT):
                xt = xp.tile([P, D_IN], f32)
                nc.sync.dma_start(xt[:], x_in[t * P : (t + 1) * P, :])
                sq = sqp.tile([P, D_IN], f32)
                ss = stats.tile([P, 1], f32)
                nc.scalar.activation(sq[:], xt[:], AF.Square, accum_out=ss[:, 0:1])
                rms = stats.tile([P, 1], f32)
                nc.scalar.activation(
                    rms[:], ss[:, 0:1], AF.Sqrt, scale=1.0 / D_IN, bias=eps_t[:, 0:1]
                )
                inv = stats.tile([P, 1], f32)
                nc.vector.reciprocal(inv[:], rms[:])
                xn = xp.tile([P, D_IN], f32)
                nc.vector.tensor_scalar_mul(xn[:], xt[:], inv[:, 0:1])
                for ko in range(KO):
                    ptr = tps.tile([P, P], f32)
                    nc.tensor.transpose(ptr[:], xn[:, ko * P : (ko + 1) * P], ident[:])
                    # cast to fp16 multiplying by gamma*scale (per-k scalar)
                    nc.vector.tensor_scalar(
                        xnT[:, ko, t * P : (t + 1) * P],
                        ptr[:],
                        gs[:, ko : ko + 1],
                        None,
                        op0=ALU.mult,
                    )

            # ---- phase B: stream weight chunks, quantize, matmul ----
            wt3 = wt_in.rearrange("(ko p) o -> p ko o", p=P)  # [128, 16, 8192]
            for oc in range(NOC):
                osl = slice(oc * OC, (oc + 1) * OC)
                wq = wqp.tile([P, KO, OC], fp16)
                for h in range(2):
                    ksl = slice(h * KH, (h + 1) * KH)
                    wr = wrawp.tile([P, KH, OC], f32)
                    nc.sync.dma_start(wr[:], wt3[:, ksl, osl])
                    # 2*wq = sign(w - tau) + sign(w + tau)   in {-2, 0, 2}
                    m1 = wmp.tile([P, KH, OC], fp16)
                    nc.scalar.activation(m1[:], wr[:], AF.Sign, bias=ntau_sb[:, 0:1])
                    m2 = wmp.tile([P, KH, OC], fp16)
                    nc.scalar.activation(m2[:], wr[:], AF.Sign, bias=tau_sb[:, 0:1])
                    nc.vector.tensor_tensor(wq[:, ksl, :], m1[:], m2[:], op=ALU.add)
                for t in range(NT):
                    ps = mps.tile([P, OC], f32)
                    for ko in range(KO):
                        nc.tensor.matmul(
                            ps[:],
                            xnT[:, ko, t * P : (t + 1) * P],
                            wq[:, ko, :],
                            start=(ko == 0),
                            stop=(ko == KO - 1),
                        )
                    ot = op.tile([P, OC], f32)
                    nc.vector.tensor_copy(ot[:], ps[:])
                    nc.sync.dma_start(out[t * P : (t + 1) * P, osl], ot[:])

    _legalize_waits(nc)
    return nc


def _ensure_ntff_hook():
    """Provide antenv.axon_hooks (missing from this image) so that
    run_bass_kernel_spmd(trace=True) can reach the libaxon NTFF profiler."""
    import types

    try:
        from antenv.axon_hooks import get_axon_ntff_profile_hook  # noqa: F401

        return True
    except ImportError:
        pass
    try:
        import antenv
        from trn_agent_boot.trn_boot import _ntff_profile_via_ctypes

        hook = _ntff_profile_via_ctypes("/opt/axon/libaxon_pjrt.so")
        mod = types.ModuleType("antenv.axon_hooks")
        _state = {"hook": hook}
        mod.set_axon_ntff_profile_hook = lambda h: _state.__setitem__("hook", h)
        mod.get_axon_ntff_profile_hook = lambda: _state["hook"]
        sys.modules["antenv.axon_hooks"] = mod
        antenv.axon_hooks = mod
        return hook is not None
    except Exception:
        return False


def _run(nc, in_maps, trace, tag):
    from concourse.bass_utils import run_bass_kernel_spmd

    kwargs = {}
    if trace and _ensure_ntff_hook():
        kwargs = dict(trace=True, trace_cores=list(range(N_CORES)))
        base = os.environ.get("BASS_PROBLEM_TRACE_DIR")
        if base:
            tdir = os.path.join(base, tag)
            os.makedirs(tdir, exist_ok=True)
            kwargs["tmpdir"] = tdir
    try:
        res = run_bass_kernel_spmd(nc, in_maps, list(range(N_CORES)), **kwargs)
    except Exception:
        if not kwargs:
            raise
        # tracing path failed; fall back to a plain run
        res = run_bass_kernel_spmd(nc, in_maps, list(range(N_CORES)))
    if trace:
        LAST_PROFILE[tag] = {
            "exec_time_ns": res.exec_time_ns,
            "mean_exec_time_ns": res.mean_exec_time_ns,
        }
    return res.results


def kernel(x, weight, gamma):
    trace = bool(int(os.environ.get("BASS_PROBLEM_TRACE", "0")))

    x = np.ascontiguousarray(np.asarray(x, dtype=np.float32))
    weight = np.ascontiguousarray(np.asarray(weight, dtype=np.float32))
    gamma = np.ascontiguousarray(np.asarray(gamma, dtype=np.float32))
    assert x.shape == (B, S, D_IN) and weight.shape == (D_OUT, D_IN)

    if "k1" not in _BUILT:
        _BUILT["k1"] = _build_scale_kernel()
    if "k2" not in _BUILT:
        _BUILT["k2"] = _build_main_kernel()

    # --- kernel 1: global mean(|w|) partials, 1/8 of the weight per core ---
    rows = D_OUT // N_CORES
    in1 = [
        {"ws": weight[c * rows : (c + 1) * rows]} for c in range(N_CORES)
    ]
    res1 = _run(_BUILT["k1"], in1, trace, "k1")
    total = np.float64(0.0)
    for c in range(N_CORES):
        total += res1[c]["partials"].astype(np.float64).sum()
    scale = np.float32(max(total / (D_OUT * D_IN), EPS_SCALE))
    tau = np.float32(0.5) * scale
    scalars = np.array([tau], dtype=np.float32)

    # --- kernel 2: RMSNorm + quantized matmul, token-parallel ---
    x_flat = x.reshape(T, D_IN)
    wT = np.ascontiguousarray(weight.T)
    in2 = [
        {
            "x": x_flat[c * TPC : (c + 1) * TPC],
            "wt": wT,
            "gamma": gamma,
            "scalars": scalars,
        }
        for c in range(N_CORES)
    ]
    res2 = _run(_BUILT["k2"], in2, trace, "k2")
    out = np.concatenate([res2[c]["out"] for c in range(N_CORES)], axis=0)
    return out.reshape(B, S, D_OUT)
